# revision 27
# baseline (speedup 1.0000x reference)
"""Quantized int8 matmul on 8 TRN2 NeuronCores.

Math: out = ((x - ZP_X) * SCALE_X) @ ((y - ZP_Y) * SCALE_Y)
Scales are folded into the fp8 operand conversion (fp8 precision is
relative, so folding changes nothing numerically): x' = (x-ZP_X)*SX
rounded to e4m3, y' = (y-ZP_Y)*SY rounded to e4m3, out = x' @ y'
accumulated in fp32 PSUM -> stored bf16 (host upcasts). fp8 rounding
gives ~1e-2 output rel err (gate 2e-2); bf16 store adds ~1e-3.
fp8 enables MatmulPerfMode.DoubleRow: 2 fp8 weights per PE cell ->
each matmul contracts 256 k-values (2 x 128) at ~1 col/cycle (216ns
per 256k x 128m x 512n MM at the warm 2.4GHz clock).

Sharding: 2D grid, M split 4 ways x N split 2 ways. Per core:
x shard [1024, 4096] (stored transposed [K, m_loc]), y shard
[4096, 2048], out [1024, 2048]. No collectives; host shards/gathers.

Per-core schedule (dense-PE design):
The original schedule loaded y in full-width k-strips, so the first
output block's matmuls were gated by the arrival of ALL 12MB of
input (8 cores x 12MB saturate HBM for ~33us) -> ~5us of PE stalls
mid-stream plus a late start. Here y is relayouted on the host into
n-block-major order ([block][p][q][pair][n]), so phase A (output
block 0, both m-quads, 8 MMs per k-step) only needs x (4MB) + y
block 0 (2MB) at a 217GB/s pace, well under the per-core HBM share.
Blocks 1-3 stream in behind on the same ring in consumption order.
The PE then runs all 512 MMs nearly back-to-back: warm-up dummies
(HAM clock-gate needs ~3.4us of busy; DMA completion semaphores
trail data by 2.5-4.5us) -> phase A chasing the block-0 stream,
with sub-pass 0 leading sub-pass 1 by two k-steps so its PSUM
evictions clear before phase B's first pass needs the banks ->
phase B free-running from SBUF. The last sub-pass runs k-contiguous
per PSUM bank so banks finish staggered and the final store is one
128KB chunk enqueued ~0.5us after the last MM (its completion
semaphore ~3us later is the end of the controllable window; the
framework teardown after it is a fixed ~7.3us).

Hardware lessons (measured, this problem):
 - exec_time = (last trace event) - (first user instruction); the
   ~5.8us engine preamble is excluded but teardown is included.
 - One SP HWDGE ring for all DMAs: two-ring splits measured WORSE
   (DMA engines round-robin rings -> per-ring bandwidth halves; the
   gpsimd/ACT rings' completion semaphores also process slower).
 - Completion semaphores trail data by 2.5-4.5us when the queue is
   deep; the early k-steps are paced by this, not by bandwidth.
 - Any PE-idle gap risks a ~430ns cold matmul right after it (HAM),
   so warm-ups are sized to ABUT the first real matmul.
 - A ~379ns matmul recurs every ~10.8us in an otherwise dense
   stream (+163ns each, firmware heartbeat?) -- accepted.

Engine split per core:
  PE     - N_WARM warm-up dummies + 512 DoubleRow matmuls
  SP     - one HWDGE ring, every DMA in consumption order: x quads,
           y block-0 chunks, y blocks 1-3, all output stores
  DVE    - x converts (int8 -> (x-zp)*s fp8), y block 1-3 converts,
           PSUM evictions for banks 0-1 of each sub-pass
  ACT    - y block-0 converts (phase A pacing), PSUM evictions for
           banks 2-3 (split so the 4 evictions of a sub-pass run on
           2 engines in parallel -> phase B starts without stalling)
  GpSimd - warm-up memsets only

Teardown note: every tile_pool context exit emits a RANGE_CLEAR +
all-engine barrier inside the measured window; this kernel uses 2
pools (SBUF, PSUM) instead of 7.
"""

import numpy as np

SCALE_X, ZP_X = 0.0215, -25
SCALE_Y, ZP_Y = 0.0176, 18
M, K, N = 4096, 4096, 4096
N_CORES = 8
M_SH, N_SH = 4, 2  # core grid: M split x N split
P = 128
NBLK = 512  # matmul moving free dim = one PSUM bank of fp32
N_WARM = 22  # PE warm-up dummy matmuls (cover HAM + DMA-sem latency);
# sized to ABUT the first real matmul: a PE-idle gap between the
# warm-ups and the real stream risks the HAM clock-gate closing and
# the first ~3.4us of real matmuls running at 1.2GHz.
WARM_N = 256  # warm-up moving free dim (granularity of the warm-up span)


def build_nc(m_loc, k, n_loc):
    from contextlib import ExitStack

    import concourse.mybir as mybir
    import concourse.tile as tile
    from concourse import bacc
    from concourse.bass import ds, ts

    fp32 = mybir.dt.float32
    bf16 = mybir.dt.bfloat16
    fp8 = mybir.dt.float8e4
    int8 = mybir.dt.int8
    Copy = mybir.ActivationFunctionType.Copy
    DoubleRow = mybir.MatmulPerfMode.DoubleRow
    add = mybir.AluOpType.add
    mult = mybir.AluOpType.mult

    MT = m_loc // P  # m tiles (8) = 2 quads of 4
    KT = k // P  # contraction tiles (32)
    Q = KT // 2  # DoubleRow k-pairs (16)
    NB = n_loc // NBLK  # output column blocks (4)
    MQ = MT // 4  # m quads (2)
    nsubs = NB * MQ  # 8 sub-passes of 4 PSUM banks each

    sx, sy = float(SCALE_X), float(SCALE_Y)
    bx, by = float(-ZP_X), float(-ZP_Y * SCALE_Y)

    nc = bacc.Bacc(None, debug=False)
    xt = nc.declare_dram_parameter("xt", [k, m_loc], int8, isOutput=False)
    # y relayouted on host to [block][p][(q, pair, n)] so each output
    # block is a contiguous 2MB region (16KB per partition)
    y = nc.declare_dram_parameter("y", [NB, P, Q * 2 * NBLK], int8, isOutput=False)
    out = nc.declare_dram_parameter("out", [m_loc, n_loc], bf16, isOutput=True)

    xt_r4 = xt.rearrange("(g b p) m -> g p b m", b=4, p=P)
    xt_r16 = xt.rearrange("(g b p) m -> g p b m", b=16, p=P)
    y_v = y.rearrange("nb p (q b n) -> nb p q b n", q=Q, b=2, n=NBLK)
    # out as [partition, m-tile, n]: one DMA stores a whole sub-pass
    out_r = out.rearrange("(t p) n -> p t n", p=P)

    # sub-pass order: phase A = block 0 (both m-quads interleaved),
    # then blocks 1-3 x (mq0, mq1)
    subs = [(0, 0), (0, 1), (1, 0), (1, 1), (2, 0), (2, 1), (3, 0), (3, 1)]

    with ExitStack() as ctx:
        tc = ctx.enter_context(tile.TileContext(nc))
        sb = ctx.enter_context(tc.tile_pool(name="sb", bufs=1))
        ps_pool = ctx.enter_context(tc.tile_pool(name="ps", bufs=8, space="PSUM"))

        # --- persistent fp8 operands
        # x^T [p, kt, m]; y [p, block, q, pair, n] (pair rows adjacent so
        # the DoubleRow moving operand reads 1024 contiguous bytes/part)
        xT = sb.tile([P, KT, m_loc], fp8, tag="xT")
        yB = sb.tile([P, NB, Q, 2, NBLK], fp8, tag="yB")

        # --- staging tiles
        xi = [
            sb.tile([P, 4, m_loc], int8, tag="xi", bufs=4, name=f"xi{g}")
            for g in range(KT // 4)
        ]
        y0s = sb.tile([P, Q, 2, NBLK], int8, tag="y0s")
        ybs = [
            sb.tile([P, Q, 2, NBLK], int8, tag="ybs", bufs=3, name=f"ybs{b}")
            for b in range(1, NB)
        ]

        # --- warm-up tiles: memsets emitted first so the warm-up
        # matmuls start the moment the preamble ends
        wm_w = sb.tile([P, P], fp8, tag="wmw")
        wm_s = sb.tile([P, WARM_N], fp8, tag="wms")
        nc.gpsimd.memset(wm_w[:], 0.0)
        nc.gpsimd.memset(wm_s[:], 0.0)

        # --- SP ring: every input DMA in consumption order, on ONE ring
        # (two-ring splits measured worse: per-ring bandwidth halves
        # under the DMA engines' round-robin, and the non-SP rings'
        # completion semaphores process noticeably slower).  The ring
        # drains FIFO, so enqueue order = arrival priority: the phase-A
        # critical path in fine chunks (x quad halves + y block-0
        # q-chunks interleaved), then the x bulk, then blocks 1-3.
        nc.sync.dma_start(xi[0][:, ds(0, 2), :], xt_r4[0][:, ds(0, 2), :])
        nc.sync.dma_start(y0s[:, ds(0, 2)], y_v[0, :, ds(0, 2)])
        nc.sync.dma_start(xi[0][:, ds(2, 2), :], xt_r4[0][:, ds(2, 2), :])
        nc.sync.dma_start(y0s[:, ds(2, 2)], y_v[0, :, ds(2, 2)])
        nc.sync.dma_start(xi[1][:], xt_r4[1])
        nc.sync.dma_start(y0s[:, ds(4, 4)], y_v[0, :, ds(4, 4)])
        nc.sync.dma_start(xi[2][:], xt_r4[2])
        nc.sync.dma_start(y0s[:, ds(8, 4)], y_v[0, :, ds(8, 4)])
        nc.sync.dma_start(xi[3][:], xt_r4[3])
        nc.sync.dma_start(y0s[:, ds(12, 4)], y_v[0, :, ds(12, 4)])
        for g in range(4, KT // 4):
            nc.sync.dma_start(xi[g][:], xt_r4[g])
        for b in range(1, NB):
            nc.sync.dma_start(ybs[b - 1][:], y_v[b])

        # --- PE warm-up: dummy matmuls on the zeroed tiles during the
        # startup DMA window so the HAM clock-gate opens before the
        # real stream begins.
        ps_warm = ps_pool.tile([P, NBLK], fp32, tag="ps", name="warm")
        for _ in range(N_WARM):
            nc.tensor.matmul(
                ps_warm[:, ds(0, WARM_N)], wm_w[:], wm_s[:], start=True, stop=True
            )

        # --- DVE: x converts (one [P,2,1024] per k-pair, ~1.2us each,
        # under the 1.73us phase-A step budget)
        for q in range(Q):
            nc.vector.tensor_scalar(
                xT[:, ds(2 * q, 2), :],
                xi[q // 2][:, ds(2 * (q % 2), 2), :],
                bx,
                sx,
                add,
                mult,
            )

        # --- ACT: y block-0 converts ([P,2,512], ~1.03us each)
        for q in range(Q):
            nc.scalar.activation(
                yB[:, 0, q], y0s[:, q], Copy, bias=by, scale=sy
            )

        def alloc_psums(s):
            return [
                ps_pool.tile([P, NBLK], fp32, tag="ps", name=f"acc_{s}_{i}")
                for i in range(4)
            ]

        def emit_sub_q(s, q, psums):
            bi, mq = subs[s]
            for mi in range(4):
                mt = mq * 4 + mi
                nc.tensor.matmul(
                    psums[mi][:],
                    xT[:, ds(2 * q, 2), ts(mt, P)],
                    yB[:, bi, q],
                    start=(q == 0),
                    stop=(q == Q - 1),
                    perf_mode=DoubleRow,
                )

        ob = {}

        def evict_sub(s, psums, banks=(0, 1, 2, 3)):
            # 4 evictions split DVE (banks 0-1) / ACT (banks 2-3) so
            # they run pairwise-parallel and the next sub-pass's
            # start=True matmuls aren't stalled on bank reuse.
            if s not in ob:
                ob[s] = sb.tile(
                    [P, 4, NBLK], bf16, tag="ob", bufs=3, name=f"ob_{s}"
                )
            for mi in banks:
                if mi < 2:
                    nc.vector.tensor_scalar_mul(
                        ob[s][:, mi, :], psums[mi][:], 1.0
                    )
                else:
                    nc.scalar.activation(ob[s][:, mi, :], psums[mi][:], Copy)

        def store_sub(s, banks=None):
            bi, mq = subs[s]
            col = bi * NBLK
            if banks is None:
                nc.sync.dma_start(
                    out_r[:, ds(mq * 4, 4), ds(col, NBLK)], ob[s][:]
                )
            else:
                for mi in banks:
                    nc.sync.dma_start(
                        out_r[:, ds(mq * 4 + mi, 1), ds(col, NBLK)],
                        ob[s][:, ds(mi, 1), :],
                    )

        # --- phase A: sub-passes 0 and 1 interleaved per k-step, chasing
        # the y block-0 / x streams.  Sub 0 LEADS by two k-steps so its
        # stop matmuls land ~1.7us before phase A ends -- its PSUM
        # evictions then complete before sub 2 needs the banks (they
        # take ~1.3us across DVE+ACT, and sub 2's start=True matmuls
        # would otherwise stall on bank reuse).
        ps0 = alloc_psums(0)
        ps1 = alloc_psums(1)
        for q in range(Q + 2):
            if q < Q:
                emit_sub_q(0, q, ps0)
            if q >= 2:
                emit_sub_q(1, q - 2, ps1)

        # --- DVE converts for y blocks 1-3 are interleaved with the
        # sub-pass emissions so the DVE FIFO order is:
        # [x converts, b1c q0-7, ev0, ev1, b1c q8-15, b2 converts, ev2,
        #  b3 converts, ev3, ev4, ev5, ev6, ev7(staggered)] -- every
        # convert batch free-runs as its DMA data lands (DVE is well
        # ahead of the PE), and every eviction lands right when its
        # sub-pass finishes.
        def emit_block_converts(bi, qs=None):
            # DVE tensor_scalar computes (in + s1) * s2 -> s1 is the RAW
            # zero point (unlike ACT's activation(scale*in + bias))
            for q in qs if qs is not None else range(Q):
                nc.vector.tensor_scalar(
                    yB[:, bi, q], ybs[bi - 1][:, q], float(-ZP_Y), sy, add, mult
                )

        emit_block_converts(1, range(0, 10))
        evict_sub(0, ps0)
        store_sub(0)
        evict_sub(1, ps1)
        store_sub(1)
        emit_block_converts(1, range(10, Q))

        # --- phase B
        psums = {0: ps0, 1: ps1}
        for s in range(2, nsubs - 1):
            psums[s] = alloc_psums(s)
            for q in range(Q):
                emit_sub_q(s, q, psums[s])
            if s == 2:
                emit_block_converts(2)
            evict_sub(s, psums[s])
            store_sub(s)
            if s == 3:
                emit_block_converts(3)

        # --- final sub-pass: k-contiguous per bank so banks finish
        # staggered ~3.5us apart; evict+store each the moment it's done.
        # The tail after the very last MM is one eviction + one 128KB
        # store instead of a 1MB batch.
        s = nsubs - 1
        psums[s] = alloc_psums(s)
        for mi in range(4):
            bi, mq = subs[s]
            for q in range(Q):
                nc.tensor.matmul(
                    psums[s][mi][:],
                    xT[:, ds(2 * q, 2), ts(mq * 4 + mi, P)],
                    yB[:, bi, q],
                    start=(q == 0),
                    stop=(q == Q - 1),
                    perf_mode=DoubleRow,
                )
            evict_sub(s, psums[s], banks=(mi,))
            if mi == 2:
                # banks 0-2 in one store; only bank 3 (the true tail)
                # needs its own minimal descriptor
                bi7, mq7 = subs[s]
                nc.sync.dma_start(
                    out_r[:, ds(mq7 * 4, 3), ds(bi7 * NBLK, NBLK)],
                    ob[s][:, ds(0, 3), :],
                )
            elif mi == 3:
                store_sub(s, banks=(3,))

    nc.compile()
    return nc


_NC_CACHE = None
LAST_RESULT = None  # BassKernelResults of the most recent run (for profiling)


def _ensure_ntff_hook():
    """concourse's trace path imports antenv.axon_hooks, which is absent
    from this container's antenv stub. Provide it (with the real libaxon
    ctypes hook when available) so tracing works -- or degrades cleanly."""
    import sys
    import types

    try:
        import antenv.axon_hooks  # noqa: F401

        return
    except ImportError:
        pass
    mod = types.ModuleType("antenv.axon_hooks")
    holder = [None]
    mod.set_axon_ntff_profile_hook = lambda h: holder.__setitem__(0, h)
    mod.get_axon_ntff_profile_hook = lambda: holder[0]
    sys.modules["antenv.axon_hooks"] = mod
    try:
        import antenv

        antenv.axon_hooks = mod
    except ImportError:
        pass
    try:
        from trn_agent_boot.trn_boot import _ntff_profile_via_ctypes

        mod.set_axon_ntff_profile_hook(
            _ntff_profile_via_ctypes("/opt/axon/libaxon_pjrt.so")
        )
    except Exception:
        pass  # no hook -> concourse logs a warning and skips tracing


def kernel(x, y):
    global _NC_CACHE, LAST_RESULT
    _ensure_ntff_hook()
    from concourse.bass_utils import run_bass_kernel_spmd

    x = np.asarray(x)
    y = np.asarray(y)
    assert x.shape == (M, K) and y.shape == (K, N), (x.shape, y.shape)
    x8 = x.astype(np.int8) if x.dtype != np.int8 else x
    y8 = y.astype(np.int8) if y.dtype != np.int8 else y

    m_loc = M // M_SH
    n_loc = N // N_SH
    Q = K // (2 * P)
    NB = n_loc // NBLK
    if _NC_CACHE is None:
        _NC_CACHE = build_nc(m_loc, K, n_loc)
    nc = _NC_CACHE

    in_maps = []
    for c in range(N_CORES):
        mi, nj = divmod(c, N_SH)
        y_sh = y8[:, nj * n_loc : (nj + 1) * n_loc]
        # [K, n_loc] -> [block][p][(q, pair, n)]
        y_re = np.ascontiguousarray(
            y_sh.reshape(Q, 2, P, NB, NBLK).transpose(3, 2, 0, 1, 4)
        ).reshape(NB, P, Q * 2 * NBLK)
        in_maps.append(
            {
                "xt": np.ascontiguousarray(x8[mi * m_loc : (mi + 1) * m_loc].T),
                "y": y_re,
            }
        )
    res = run_bass_kernel_spmd(nc, in_maps, core_ids=list(range(N_CORES)))
    LAST_RESULT = res
    full = np.empty((M, N), dtype=np.float32)
    for c in range(N_CORES):
        mi, nj = divmod(c, N_SH)
        full[mi * m_loc : (mi + 1) * m_loc, nj * n_loc : (nj + 1) * n_loc] = (
            np.asarray(res.results[c]["out"]).astype(np.float32)
        )
    return full


# revision 28
# speedup vs baseline: 1.0165x; 1.0165x over previous
"""Quantized int8 matmul on 8 TRN2 NeuronCores.

Math: out = ((x - ZP_X) * SCALE_X) @ ((y - ZP_Y) * SCALE_Y)
Scales are folded into the fp8 operand conversion (fp8 precision is
relative, so folding changes nothing numerically): x' = (x-ZP_X)*SX
rounded to e4m3, y' = (y-ZP_Y)*SY rounded to e4m3, out = x' @ y'
accumulated in fp32 PSUM -> stored bf16 (host upcasts). fp8 rounding
gives ~1e-2 output rel err (gate 2e-2); bf16 store adds ~1e-3.
fp8 enables MatmulPerfMode.DoubleRow: 2 fp8 weights per PE cell ->
each matmul contracts 256 k-values (2 x 128) at ~1 col/cycle (216ns
per 256k x 128m x 512n MM at the warm 2.4GHz clock).

Sharding: 2D grid, M split 4 ways x N split 2 ways. Per core:
x shard [1024, 4096] (stored transposed [K, m_loc]), y shard
[4096, 2048], out [1024, 2048]. No collectives; host shards/gathers.

Per-core schedule (dense-PE design):
The original schedule loaded y in full-width k-strips, so the first
output block's matmuls were gated by the arrival of ALL 12MB of
input (8 cores x 12MB saturate HBM for ~33us) -> ~5us of PE stalls
mid-stream plus a late start. Here y is relayouted on the host into
n-block-major order ([block][p][q][pair][n]), so phase A (output
block 0, both m-quads, 8 MMs per k-step) only needs x (4MB) + y
block 0 (2MB) at a 217GB/s pace, well under the per-core HBM share.
Blocks 1-3 stream in behind on the same ring in consumption order.
The PE then runs all 512 MMs nearly back-to-back: warm-up dummies
(HAM clock-gate needs ~3.4us of busy; DMA completion semaphores
trail data by 2.5-4.5us) -> phase A chasing the block-0 stream,
with sub-pass 0 leading sub-pass 1 by two k-steps so its PSUM
evictions clear before phase B's first pass needs the banks ->
phase B free-running from SBUF. The last sub-pass runs k-contiguous
per PSUM bank so banks finish staggered and the final store is one
128KB chunk enqueued ~0.5us after the last MM (its completion
semaphore ~3us later is the end of the controllable window; the
framework teardown after it is a fixed ~7.3us).

Hardware lessons (measured, this problem):
 - exec_time = (last trace event) - (first user instruction); the
   ~5.8us engine preamble is excluded but teardown is included.
 - One SP HWDGE ring for all DMAs: two-ring splits measured WORSE
   (DMA engines round-robin rings -> per-ring bandwidth halves; the
   gpsimd/ACT rings' completion semaphores also process slower).
 - Completion semaphores trail data by 2.5-4.5us when the queue is
   deep; the early k-steps are paced by this, not by bandwidth.
 - Any PE-idle gap risks a ~430ns cold matmul right after it (HAM),
   so warm-ups are sized to ABUT the first real matmul.
 - A ~379ns matmul recurs every ~10.8us in an otherwise dense
   stream (+163ns each, firmware heartbeat?) -- accepted.

Engine split per core:
  PE     - N_WARM warm-up dummies + 512 DoubleRow matmuls
  SP     - one HWDGE ring, every DMA in consumption order: x quads,
           y block-0 chunks, y blocks 1-3, all output stores
  DVE    - x converts (int8 -> (x-zp)*s fp8), y block 1-3 converts,
           PSUM evictions for banks 0-1 of each sub-pass
  ACT    - y block-0 converts (phase A pacing), PSUM evictions for
           banks 2-3 (split so the 4 evictions of a sub-pass run on
           2 engines in parallel -> phase B starts without stalling)
  GpSimd - warm-up memsets only

Teardown note: every tile_pool context exit emits a RANGE_CLEAR +
all-engine barrier inside the measured window; this kernel uses 2
pools (SBUF, PSUM) instead of 7.
"""

import numpy as np

SCALE_X, ZP_X = 0.0215, -25
SCALE_Y, ZP_Y = 0.0176, 18
M, K, N = 4096, 4096, 4096
N_CORES = 8
M_SH, N_SH = 4, 2  # core grid: M split x N split
P = 128
NBLK = 512  # matmul moving free dim = one PSUM bank of fp32
N_WARM = 26  # PE warm-up dummy matmuls (cover HAM + DMA-sem latency);
# sized to ABUT the first real matmul: a PE-idle gap between the
# warm-ups and the real stream risks the HAM clock-gate closing and
# the first ~3.4us of real matmuls running at 1.2GHz.
WARM_N = 256  # warm-up moving free dim (granularity of the warm-up span)


def build_nc(m_loc, k, n_loc):
    from contextlib import ExitStack

    import concourse.mybir as mybir
    import concourse.tile as tile
    from concourse import bacc
    from concourse.bass import ds, ts

    fp32 = mybir.dt.float32
    bf16 = mybir.dt.bfloat16
    fp8 = mybir.dt.float8e4
    int8 = mybir.dt.int8
    Copy = mybir.ActivationFunctionType.Copy
    DoubleRow = mybir.MatmulPerfMode.DoubleRow
    add = mybir.AluOpType.add
    mult = mybir.AluOpType.mult

    MT = m_loc // P  # m tiles (8) = 2 quads of 4
    KT = k // P  # contraction tiles (32)
    Q = KT // 2  # DoubleRow k-pairs (16)
    NB = n_loc // NBLK  # output column blocks (4)
    MQ = MT // 4  # m quads (2)
    nsubs = NB * MQ  # 8 sub-passes of 4 PSUM banks each

    sx, sy = float(SCALE_X), float(SCALE_Y)
    bx, by = float(-ZP_X), float(-ZP_Y * SCALE_Y)

    nc = bacc.Bacc(None, debug=False)
    xt = nc.declare_dram_parameter("xt", [k, m_loc], int8, isOutput=False)
    # y relayouted on host to [block][p][(q, pair, n)] so each output
    # block is a contiguous 2MB region (16KB per partition)
    y = nc.declare_dram_parameter("y", [NB, P, Q * 2 * NBLK], int8, isOutput=False)
    out = nc.declare_dram_parameter("out", [m_loc, n_loc], bf16, isOutput=True)

    xt_r4 = xt.rearrange("(g b p) m -> g p b m", b=4, p=P)
    xt_r16 = xt.rearrange("(g b p) m -> g p b m", b=16, p=P)
    y_v = y.rearrange("nb p (q b n) -> nb p q b n", q=Q, b=2, n=NBLK)
    # out as [partition, m-tile, n]: one DMA stores a whole sub-pass
    out_r = out.rearrange("(t p) n -> p t n", p=P)

    # sub-pass order: phase A = block 0 (both m-quads interleaved),
    # then blocks 1-3 x (mq0, mq1)
    subs = [(0, 0), (0, 1), (1, 0), (1, 1), (2, 0), (2, 1), (3, 0), (3, 1)]

    with ExitStack() as ctx:
        tc = ctx.enter_context(tile.TileContext(nc))
        sb = ctx.enter_context(tc.tile_pool(name="sb", bufs=1))
        ps_pool = ctx.enter_context(tc.tile_pool(name="ps", bufs=8, space="PSUM"))

        # --- persistent fp8 operands
        # x^T [p, kt, m]; y [p, block, q, pair, n] (pair rows adjacent so
        # the DoubleRow moving operand reads 1024 contiguous bytes/part)
        xT = sb.tile([P, KT, m_loc], fp8, tag="xT")
        yB = sb.tile([P, NB, Q, 2, NBLK], fp8, tag="yB")

        # --- staging tiles
        xi = [
            sb.tile([P, 4, m_loc], int8, tag="xi", bufs=4, name=f"xi{g}")
            for g in range(KT // 4)
        ]
        y0s = sb.tile([P, Q, 2, NBLK], int8, tag="y0s")
        ybs = [
            sb.tile([P, Q, 2, NBLK], int8, tag="ybs", bufs=3, name=f"ybs{b}")
            for b in range(1, NB)
        ]

        # --- warm-up tiles: memsets emitted first so the warm-up
        # matmuls start the moment the preamble ends
        wm_w = sb.tile([P, P], fp8, tag="wmw")
        wm_s = sb.tile([P, WARM_N], fp8, tag="wms")
        nc.gpsimd.memset(wm_w[:], 0.0)
        nc.gpsimd.memset(wm_s[:], 0.0)

        # --- SP ring: every input DMA in consumption order, on ONE ring
        # (two-ring splits measured worse: per-ring bandwidth halves
        # under the DMA engines' round-robin, and the non-SP rings'
        # completion semaphores process noticeably slower).  The ring
        # drains FIFO, so enqueue order = arrival priority: the phase-A
        # critical path in fine chunks (x quad halves + y block-0
        # q-chunks interleaved), then the x bulk, then blocks 1-3.
        nc.sync.dma_start(xi[0][:, ds(0, 2), :], xt_r4[0][:, ds(0, 2), :])
        nc.sync.dma_start(y0s[:, ds(0, 2)], y_v[0, :, ds(0, 2)])
        nc.sync.dma_start(xi[0][:, ds(2, 2), :], xt_r4[0][:, ds(2, 2), :])
        nc.sync.dma_start(y0s[:, ds(2, 2)], y_v[0, :, ds(2, 2)])
        nc.sync.dma_start(xi[1][:], xt_r4[1])
        nc.sync.dma_start(y0s[:, ds(4, 4)], y_v[0, :, ds(4, 4)])
        nc.sync.dma_start(xi[2][:], xt_r4[2])
        nc.sync.dma_start(y0s[:, ds(8, 4)], y_v[0, :, ds(8, 4)])
        nc.sync.dma_start(xi[3][:], xt_r4[3])
        nc.sync.dma_start(y0s[:, ds(12, 4)], y_v[0, :, ds(12, 4)])
        for g in range(4, KT // 4):
            nc.sync.dma_start(xi[g][:], xt_r4[g])
        for b in range(1, NB):
            nc.sync.dma_start(ybs[b - 1][:], y_v[b])

        # --- PE warm-up: dummy matmuls on the zeroed tiles during the
        # startup DMA window so the HAM clock-gate opens before the
        # real stream begins.
        ps_warm = ps_pool.tile([P, NBLK], fp32, tag="ps", name="warm")
        for _ in range(N_WARM):
            nc.tensor.matmul(
                ps_warm[:, ds(0, WARM_N)], wm_w[:], wm_s[:], start=True, stop=True
            )

        # --- DVE: x converts (one [P,2,1024] per k-pair, ~1.2us each,
        # under the 1.73us phase-A step budget)
        for q in range(Q):
            nc.vector.tensor_scalar(
                xT[:, ds(2 * q, 2), :],
                xi[q // 2][:, ds(2 * (q % 2), 2), :],
                bx,
                sx,
                add,
                mult,
            )

        # --- ACT: y block-0 converts ([P,2,512], ~1.03us each)
        for q in range(Q):
            nc.scalar.activation(
                yB[:, 0, q], y0s[:, q], Copy, bias=by, scale=sy
            )

        def alloc_psums(s):
            return [
                ps_pool.tile([P, NBLK], fp32, tag="ps", name=f"acc_{s}_{i}")
                for i in range(4)
            ]

        def emit_sub_q(s, q, psums):
            bi, mq = subs[s]
            for mi in range(4):
                mt = mq * 4 + mi
                nc.tensor.matmul(
                    psums[mi][:],
                    xT[:, ds(2 * q, 2), ts(mt, P)],
                    yB[:, bi, q],
                    start=(q == 0),
                    stop=(q == Q - 1),
                    perf_mode=DoubleRow,
                )

        ob = {}

        def evict_sub(s, psums, banks=(0, 1, 2, 3)):
            # 4 evictions split DVE (banks 0-1) / ACT (banks 2-3) so
            # they run pairwise-parallel and the next sub-pass's
            # start=True matmuls aren't stalled on bank reuse.
            if s not in ob:
                ob[s] = sb.tile(
                    [P, 4, NBLK], bf16, tag="ob", bufs=3, name=f"ob_{s}"
                )
            for mi in banks:
                if mi < 2:
                    nc.vector.tensor_scalar_mul(
                        ob[s][:, mi, :], psums[mi][:], 1.0
                    )
                else:
                    nc.scalar.activation(ob[s][:, mi, :], psums[mi][:], Copy)

        def store_sub(s, banks=None):
            bi, mq = subs[s]
            col = bi * NBLK
            if banks is None:
                nc.sync.dma_start(
                    out_r[:, ds(mq * 4, 4), ds(col, NBLK)], ob[s][:]
                )
            else:
                for mi in banks:
                    nc.sync.dma_start(
                        out_r[:, ds(mq * 4 + mi, 1), ds(col, NBLK)],
                        ob[s][:, ds(mi, 1), :],
                    )

        # --- phase A: sub-passes 0 and 1 interleaved per k-step, chasing
        # the y block-0 / x streams.  Sub 0 LEADS by two k-steps so its
        # stop matmuls land ~1.7us before phase A ends -- its PSUM
        # evictions then complete before sub 2 needs the banks (they
        # take ~1.3us across DVE+ACT, and sub 2's start=True matmuls
        # would otherwise stall on bank reuse).
        ps0 = alloc_psums(0)
        ps1 = alloc_psums(1)
        for q in range(Q + 2):
            if q < Q:
                emit_sub_q(0, q, ps0)
            if q >= 2:
                emit_sub_q(1, q - 2, ps1)

        # --- DVE converts for y blocks 1-3 are interleaved with the
        # sub-pass emissions so the DVE FIFO order is:
        # [x converts, b1c q0-7, ev0, ev1, b1c q8-15, b2 converts, ev2,
        #  b3 converts, ev3, ev4, ev5, ev6, ev7(staggered)] -- every
        # convert batch free-runs as its DMA data lands (DVE is well
        # ahead of the PE), and every eviction lands right when its
        # sub-pass finishes.
        def emit_block_converts(bi, qs=None):
            # DVE tensor_scalar computes (in + s1) * s2 -> s1 is the RAW
            # zero point (unlike ACT's activation(scale*in + bias))
            for q in qs if qs is not None else range(Q):
                nc.vector.tensor_scalar(
                    yB[:, bi, q], ybs[bi - 1][:, q], float(-ZP_Y), sy, add, mult
                )

        emit_block_converts(1, range(0, 10))
        evict_sub(0, ps0)
        store_sub(0)
        evict_sub(1, ps1)
        store_sub(1)
        emit_block_converts(1, range(10, Q))

        # --- phase B
        psums = {0: ps0, 1: ps1}
        for s in range(2, nsubs - 1):
            psums[s] = alloc_psums(s)
            for q in range(Q):
                emit_sub_q(s, q, psums[s])
            if s == 2:
                emit_block_converts(2)
            evict_sub(s, psums[s])
            store_sub(s)
            if s == 3:
                emit_block_converts(3)

        # --- final sub-pass: k-contiguous per bank so banks finish
        # staggered ~3.5us apart; evict+store each the moment it's done.
        # The tail after the very last MM is one eviction + one 128KB
        # store instead of a 1MB batch.
        s = nsubs - 1
        psums[s] = alloc_psums(s)
        for mi in range(4):
            bi, mq = subs[s]
            for q in range(Q):
                nc.tensor.matmul(
                    psums[s][mi][:],
                    xT[:, ds(2 * q, 2), ts(mq * 4 + mi, P)],
                    yB[:, bi, q],
                    start=(q == 0),
                    stop=(q == Q - 1),
                    perf_mode=DoubleRow,
                )
            evict_sub(s, psums[s], banks=(mi,))
            if mi == 2:
                # banks 0-2 in one store; only bank 3 (the true tail)
                # needs its own minimal descriptor
                bi7, mq7 = subs[s]
                nc.sync.dma_start(
                    out_r[:, ds(mq7 * 4, 3), ds(bi7 * NBLK, NBLK)],
                    ob[s][:, ds(0, 3), :],
                )
            elif mi == 3:
                store_sub(s, banks=(3,))

    nc.compile()
    return nc


_NC_CACHE = None
LAST_RESULT = None  # BassKernelResults of the most recent run (for profiling)


def _ensure_ntff_hook():
    """concourse's trace path imports antenv.axon_hooks, which is absent
    from this container's antenv stub. Provide it (with the real libaxon
    ctypes hook when available) so tracing works -- or degrades cleanly."""
    import sys
    import types

    try:
        import antenv.axon_hooks  # noqa: F401

        return
    except ImportError:
        pass
    mod = types.ModuleType("antenv.axon_hooks")
    holder = [None]
    mod.set_axon_ntff_profile_hook = lambda h: holder.__setitem__(0, h)
    mod.get_axon_ntff_profile_hook = lambda: holder[0]
    sys.modules["antenv.axon_hooks"] = mod
    try:
        import antenv

        antenv.axon_hooks = mod
    except ImportError:
        pass
    try:
        from trn_agent_boot.trn_boot import _ntff_profile_via_ctypes

        mod.set_axon_ntff_profile_hook(
            _ntff_profile_via_ctypes("/opt/axon/libaxon_pjrt.so")
        )
    except Exception:
        pass  # no hook -> concourse logs a warning and skips tracing


def kernel(x, y):
    global _NC_CACHE, LAST_RESULT
    _ensure_ntff_hook()
    from concourse.bass_utils import run_bass_kernel_spmd

    x = np.asarray(x)
    y = np.asarray(y)
    assert x.shape == (M, K) and y.shape == (K, N), (x.shape, y.shape)
    x8 = x.astype(np.int8) if x.dtype != np.int8 else x
    y8 = y.astype(np.int8) if y.dtype != np.int8 else y

    m_loc = M // M_SH
    n_loc = N // N_SH
    Q = K // (2 * P)
    NB = n_loc // NBLK
    if _NC_CACHE is None:
        _NC_CACHE = build_nc(m_loc, K, n_loc)
    nc = _NC_CACHE

    in_maps = []
    for c in range(N_CORES):
        mi, nj = divmod(c, N_SH)
        y_sh = y8[:, nj * n_loc : (nj + 1) * n_loc]
        # [K, n_loc] -> [block][p][(q, pair, n)]
        y_re = np.ascontiguousarray(
            y_sh.reshape(Q, 2, P, NB, NBLK).transpose(3, 2, 0, 1, 4)
        ).reshape(NB, P, Q * 2 * NBLK)
        in_maps.append(
            {
                "xt": np.ascontiguousarray(x8[mi * m_loc : (mi + 1) * m_loc].T),
                "y": y_re,
            }
        )
    res = run_bass_kernel_spmd(nc, in_maps, core_ids=list(range(N_CORES)))
    LAST_RESULT = res
    full = np.empty((M, N), dtype=np.float32)
    for c in range(N_CORES):
        mi, nj = divmod(c, N_SH)
        full[mi * m_loc : (mi + 1) * m_loc, nj * n_loc : (nj + 1) * n_loc] = (
            np.asarray(res.results[c]["out"]).astype(np.float32)
        )
    return full
